# revision 1
# baseline (speedup 1.0000x reference)
"""Per-sample depthwise 7x7 SAME cross-correlation on 8 trn2 NeuronCores.

Problem: inputs [32,128,128,128] (B,H,W,C), kernels [32,7,7,128] (B,KH,KW,C).
out[b,y,x,c] = sum_{i,j} inputs[b, y+i-3, x+j-3, c] * kernels[b,i,j,c]

Strategy (pure data parallel, batch sharded 4 samples/core):
  - Host: transpose to channel-major [b, c, y, x], zero-pad spatially to
    134x134 so every tap is a plain shifted AP read (SAME padding built in).
  - On-chip layout: C=128 on partitions, (y, x) in the free dim. The
    per-(b,c) kernel tap value is a per-partition scalar, so each tap is one
    fused multiply-accumulate: scalar_tensor_tensor(acc = x_shift * w + acc).
  - Taps are split across VectorE (fused MACs, 32 taps) and GpSimdE (adds of
    per-partition-scaled products that ScalarE produces, 17 taps), so all
    three elementwise-capable engines run concurrently; the two partial
    accumulators are merged on VectorE and DMA'd out channel-major.
  - Host transposes the gathered result back to [B,H,W,C].

Why not the TensorEngine: a depthwise conv with per-(b,c) kernels has no
shared contraction — any matmul formulation either needs per-channel banded
weight matrices (whose on-chip materialization costs more than the conv
itself: 3584 128x128 bands vs 512 images) or wastes >=127/128 of the array
on diagonal weights. The elementwise path on VectorE is the real roofline.
"""

import numpy as np

import concourse.bass as bass
import concourse.tile as tile
from concourse import bacc, mybir
from concourse.bass_utils import run_bass_kernel_spmd

B, H, W, C = 32, 128, 128, 128
KH = KW = 7
PAD = 3
N_CORES = 8
BPC = B // N_CORES  # samples per core
HP, WP = H + 2 * PAD, W + 2 * PAD  # 134, 134
SLAB = 32  # output rows per compute slab
N_SLABS = H // SLAB

# Tap split across the engines (tuned via cost-model + HW sweep).
_ALL_TAPS = [(i, j) for i in range(KH) for j in range(KW)]
N_GP_TAPS = 18
_GP_TAPS = _ALL_TAPS[:N_GP_TAPS]
_DVE_TAPS = _ALL_TAPS[N_GP_TAPS:]
# Independent VectorE accumulator chains: back-to-back dependent DVE ops pay
# a pipeline DRAIN ~= op duration (measured 2.15x); interleaved independent
# chains overlap it (measured 1.88x recovery on a DVE-only variant).
N_DVE_CHAINS = 3

_PROGRAM_CACHE = {}


def _build_program(repeat=1):
    f32 = mybir.dt.float32
    nc = bacc.Bacc("TRN2", target_bir_lowering=False, debug=False)
    x_h = nc.dram_tensor("x", [BPC, C, HP, WP], f32, kind="ExternalInput")
    w_h = nc.dram_tensor("w", [BPC, C, KH * KW], f32, kind="ExternalInput")
    o_h = nc.dram_tensor("o", [BPC, C, H, W], f32, kind="ExternalOutput")
    x, w, o = x_h.ap(), w_h.ap(), o_h.ap()

    with tile.TileContext(nc) as tc:
        with (
            tc.tile_pool(name="wpool", bufs=1) as wpool,
            tc.tile_pool(name="xpool", bufs=3) as xpool,
            tc.tile_pool(name="accd0", bufs=2) as accd0p,
            tc.tile_pool(name="accdx", bufs=1) as accdxp,
            tc.tile_pool(name="accg", bufs=2) as accgp,
            tc.tile_pool(name="accg1", bufs=1) as accg1p,
            tc.tile_pool(name="tmp", bufs=2) as tmpp,
        ):
            wall = wpool.tile([C, BPC, KH * KW], f32)
            for b in range(BPC):
                nc.sync.dma_start(out=wall[:, b, :], in_=w[b])

            for b, s in [
                (b, s)
                for _ in range(repeat)
                for b in range(BPC)
                for s in range(N_SLABS)
            ]:
                if True:
                    y0 = s * SLAB
                    xt = xpool.tile([C, SLAB + 2 * PAD, WP], f32)
                    nc.sync.dma_start(out=xt, in_=x[b, :, y0 : y0 + SLAB + 2 * PAD, :])

                    dacc = [
                        (accd0p if ch == 0 else accdxp).tile(
                            [C, SLAB, W], f32, name=f"dacc{ch}", tag=f"dacc{ch}"
                        )
                        for ch in range(N_DVE_CHAINS)
                    ]
                    if _GP_TAPS:
                        acc_g = accgp.tile([C, SLAB, W], f32)
                        acc_g1 = accg1p.tile([C, SLAB, W], f32)
                        gacc = [acc_g, acc_g1]
                    else:
                        acc_g = None

                    started = [False] * N_DVE_CHAINS
                    for t, (i, j) in enumerate(_DVE_TAPS):
                        ch = t % N_DVE_CHAINS
                        xin = xt[:, i : i + SLAB, j : j + W]
                        wsc = wall[:, b, i * KW + j : i * KW + j + 1]
                        if not started[ch]:
                            nc.vector.tensor_scalar_mul(dacc[ch], xin, wsc)
                            started[ch] = True
                        else:
                            nc.vector.scalar_tensor_tensor(
                                out=dacc[ch], in0=xin, scalar=wsc, in1=dacc[ch],
                                op0=mybir.AluOpType.mult, op1=mybir.AluOpType.add,
                            )
                    # GpSimd side: 2 interleaved accumulator chains (same
                    # drain-overlap rationale as the VectorE chains); ScalarE
                    # seeds each chain and produces every product.
                    gstarted = [False, False]
                    for t, (i, j) in enumerate(_GP_TAPS):
                        gch = t % 2
                        xin = xt[:, i : i + SLAB, j : j + W]
                        wsc = wall[:, b, i * KW + j : i * KW + j + 1]
                        if not gstarted[gch]:
                            nc.scalar.mul(gacc[gch], xin, wsc)
                            gstarted[gch] = True
                        else:
                            prod = tmpp.tile([C, SLAB, W], f32)
                            nc.scalar.mul(prod, xin, wsc)
                            nc.gpsimd.tensor_add(gacc[gch], gacc[gch], prod)
                    nc.gpsimd.tensor_add(acc_g, acc_g, acc_g1)
                    # Tree merge: first level is two INDEPENDENT adds whose
                    # pipeline drains overlap; only the final add is serial.
                    if N_DVE_CHAINS == 3 and acc_g is not None:
                        nc.vector.tensor_add(dacc[0], dacc[0], dacc[1])
                        nc.vector.tensor_add(dacc[2], dacc[2], acc_g)
                        nc.vector.tensor_add(dacc[0], dacc[0], dacc[2])
                    else:
                        for ch in range(1, N_DVE_CHAINS):
                            nc.vector.tensor_add(dacc[0], dacc[0], dacc[ch])
                        if acc_g is not None:
                            nc.vector.tensor_add(dacc[0], dacc[0], acc_g)
                    nc.sync.dma_start(out=o[b, :, y0 : y0 + SLAB, :], in_=dacc[0])

    nc.compile()
    return nc


def _get_program():
    if "nc" not in _PROGRAM_CACHE:
        _PROGRAM_CACHE["nc"] = _build_program()
    return _PROGRAM_CACHE["nc"]


def _prep_inputs(inputs, kernels):
    """Host-side shard + layout transform. Returns per-core input maps."""
    xt = _PROGRAM_CACHE.get("xt")
    if xt is None:
        xt = np.zeros((B, C, HP, WP), np.float32)
        _PROGRAM_CACHE["xt"] = xt
    xt[:, :, PAD : PAD + H, PAD : PAD + W] = np.transpose(inputs, (0, 3, 1, 2))
    wt = np.ascontiguousarray(
        np.transpose(kernels, (0, 3, 1, 2)).reshape(B, C, KH * KW)
    )
    in_maps = []
    for k in range(N_CORES):
        sl = slice(k * BPC, (k + 1) * BPC)
        in_maps.append({"x": xt[sl], "w": wt[sl]})
    return in_maps


def _gather_output(results):
    full = np.concatenate([r["o"] for r in results], axis=0)  # [B, C, H, W]
    return np.ascontiguousarray(np.transpose(full, (0, 2, 3, 1)))


def run_spmd(inputs, kernels, **spmd_kwargs):
    """Run on all 8 cores; returns (output, BassKernelResults)."""
    nc = _get_program()
    in_maps = _prep_inputs(np.asarray(inputs), np.asarray(kernels))
    res = run_bass_kernel_spmd(nc, in_maps, list(range(N_CORES)), **spmd_kwargs)
    return _gather_output(res.results), res


def kernel(inputs, kernels):
    out, _ = run_spmd(inputs, kernels)
    return out



# revision 2
# speedup vs baseline: 2.7069x; 2.7069x over previous
"""Per-sample depthwise 7x7 SAME cross-correlation on 8 trn2 NeuronCores.

Problem: inputs [32,128,128,128] (B,H,W,C), kernels [32,7,7,128] (B,KH,KW,C).
out[b,y,x,c] = sum_{i,j} inputs[b, y+i-3, x+j-3, c] * kernels[b,i,j,c]

Strategy (pure data parallel, 4 samples/core; all four compute engines used):
  - Host: channel-major fp16 zero-padded images [B, C, 134, 134]; per-tap
    fp32 scalars [B, C, 49]; fp16 output (converted back to fp32 on host).
    fp16 in/out halves DMA and enables the DVE 2-byte fast paths; products
    are exact to ~1e-3 which is far inside the 2e-2 gate.
  - The 49 taps are split across three concurrent compute paths:
      * TensorE (28 taps): out[c,:] += w_t[c] * xshift[c,:] as a matmul with
        a per-(sample,tap) DIAGONAL stationary matrix diag(kernels[b,:,i,j])
        and the shifted image as moving data; PSUM accumulates all taps in
        fp32. Diagonals are built on the fly (DVE: eye * w per-partition
        scalar, 49 small ops per sample). 512-px chunks (1 PSUM bank each).
      * DVE (14 taps): product passes via tensor_scalar_mul (4x fp16 mode)
        plus tensor_tensor adds (2x fp16 mode) into 2 independent fp16
        accumulator chains; cheaper than the fused scalar_tensor_tensor
        MAC which has no DVE fast modes.
      * GpSimd (7 taps): tensor_tensor adds of DVE-produced products into a
        third fp16 accumulator (the HW Pool engine only accepts
        TensorTensor-class ops, not TensorScalarPtr).
    The two side accumulators are absorbed into PSUM by identity matmuls
    and ScalarE evicts PSUM (fp32) into fp16 output tiles for DMA.
  - 16-row slabs: 4 PSUM banks per slab, double-buffered across slabs so
    eviction overlaps the next slab's matmuls.
"""

import numpy as np

import concourse.bass as bass
import concourse.tile as tile
from concourse import bacc, mybir
from concourse.bass_utils import run_bass_kernel_spmd

B, H, W, C = 32, 128, 128, 128
KH = KW = 7
PAD = 3
N_CORES = 8
BPC = B // N_CORES
HP, WP = H + 2 * PAD, W + 2 * PAD  # 134
SLAB = 16
N_SLABS = H // SLAB  # 8 per sample
ROWS = SLAB + 2 * PAD  # 22 input rows per slab
CHUNK_ROWS = 4  # 4*128 = 512 px = one PSUM bank
NCHUNK = SLAB // CHUNK_ROWS  # 4

# Tap split across engines (model-balanced: PE 241.9ns/chunk, DVE
# 611+1144ns/slab-pass, Pool 4158ns/add).
_ALL = [(i, j) for i in range(KH) for j in range(KW)]
N_PE, N_DVE, N_POOL = 28, 14, 7
PE_TAPS = _ALL[:N_PE]
DVE_TAPS = _ALL[N_PE : N_PE + N_DVE]
POOL_TAPS = _ALL[N_PE + N_DVE :]

f32 = mybir.dt.float32
f16 = mybir.dt.float16
A = mybir.AluOpType

_PROGRAM_CACHE = {}


def _build_program():
    nc = bacc.Bacc("TRN2", target_bir_lowering=False, debug=False)
    x_h = nc.dram_tensor("x", [BPC, C, HP, WP], f16, kind="ExternalInput")
    w_h = nc.dram_tensor("w", [BPC, C, KH * KW], f32, kind="ExternalInput")
    eye_h = nc.dram_tensor("eye", [C, C], f16, kind="ExternalInput")
    o_h = nc.dram_tensor("o", [BPC, C, H, W], f16, kind="ExternalOutput")
    x, w, eye, o = x_h.ap(), w_h.ap(), eye_h.ap(), o_h.ap()

    with tile.TileContext(nc) as tc:
        with (
            tc.tile_pool(name="const", bufs=1) as constp,
            tc.tile_pool(name="xp", bufs=3) as xp,
            tc.tile_pool(name="diagp", bufs=2) as diagp,
            tc.tile_pool(name="daccp", bufs=2) as daccp,
            tc.tile_pool(name="paccp", bufs=2) as paccp,
            tc.tile_pool(name="prodp", bufs=3) as prodp,
            tc.tile_pool(name="ppp", bufs=3) as ppp,
            tc.tile_pool(name="outp", bufs=2) as outp,
            tc.psum_pool(name="ps", bufs=2) as psp,
        ):
            eye_t = constp.tile([C, C], f16)
            wall = constp.tile([C, BPC, KH * KW], f32)
            nc.sync.dma_start(out=eye_t, in_=eye)
            for b in range(BPC):
                nc.sync.dma_start(out=wall[:, b, :], in_=w[b])

            def build_diags(b):
                """49 diagonal stationary matrices for sample b."""
                dt = diagp.tile(
                    [C, KH * KW, C], f16, name=f"diag{b}", tag=f"diag{b % 2}"
                )
                for t in range(KH * KW):
                    nc.vector.tensor_scalar_mul(
                        dt[:, t, :], eye_t, wall[:, b, t : t + 1]
                    )
                return dt

            diag_tiles = {0: build_diags(0)}

            for b in range(BPC):
                for s in range(N_SLABS):
                    y0 = s * SLAB
                    xt = xp.tile([C, ROWS, WP], f16)
                    nc.sync.dma_start(out=xt, in_=x[b, :, y0 : y0 + ROWS, :])
                    dt = diag_tiles[b]

                    # --- PE diag taps, tap-outer for stationary reuse ---
                    pst = [
                        psp.tile([C, 512], f32, name=f"ps{q}", tag=f"ps{q}")
                        for q in range(NCHUNK)
                    ]
                    for ti, (i, j) in enumerate(PE_TAPS):
                        t = i * KW + j
                        for q in range(NCHUNK):
                            r0 = i + q * CHUNK_ROWS
                            rhs = xt[:, r0 : r0 + CHUNK_ROWS, j : j + W]
                            nc.tensor.matmul(
                                pst[q], dt[:, t, :], rhs,
                                start=(ti == 0), stop=False,
                            )

                    # --- DVE + Pool paths ---
                    pacc = paccp.tile([C, SLAB, W], f16, name="pacc", tag="pacc")
                    dacc = [
                        daccp.tile([C, SLAB, W], f16, name=f"dacc{ch}", tag=f"dacc{ch}")
                        for ch in range(2)
                    ]

                    def xin(i, j):
                        return xt[:, i : i + SLAB, j : j + W]

                    # pool seed + dacc seeds
                    (pi, pj) = POOL_TAPS[0]
                    nc.vector.tensor_scalar_mul(
                        pacc, xin(pi, pj), wall[:, b, pi * KW + pj : pi * KW + pj + 1]
                    )
                    for ch in range(2):
                        (di, dj) = DVE_TAPS[ch]
                        nc.vector.tensor_scalar_mul(
                            dacc[ch], xin(di, dj),
                            wall[:, b, di * KW + dj : di * KW + dj + 1],
                        )

                    n_iter = (N_DVE - 2) // 2  # dacc chain steps
                    pool_rest = POOL_TAPS[1:]
                    for k in range(n_iter):
                        # one pool product per chain step keeps Pool fed
                        if k < len(pool_rest):
                            (i, j) = pool_rest[k]
                            pp = ppp.tile(
                                [C, SLAB, W], f16, name=f"pp{k}", tag=f"pp{k % 3}"
                            )
                            nc.vector.tensor_scalar_mul(
                                pp, xin(i, j), wall[:, b, i * KW + j : i * KW + j + 1]
                            )
                            nc.gpsimd.tensor_tensor(
                                out=pacc, in0=pp, in1=pacc, op=A.add
                            )
                        prods = []
                        for ch in range(2):
                            (i, j) = DVE_TAPS[2 + 2 * k + ch]
                            pr = prodp.tile(
                                [C, SLAB, W], f16, name=f"pr{ch}", tag=f"pr{ch}"
                            )
                            nc.vector.tensor_scalar_mul(
                                pr, xin(i, j), wall[:, b, i * KW + j : i * KW + j + 1]
                            )
                            prods.append(pr)
                        for ch in range(2):
                            nc.vector.tensor_tensor(
                                out=dacc[ch], in0=prods[ch], in1=dacc[ch], op=A.add
                            )
                    # leftover pool products (if any)
                    for k in range(n_iter, len(pool_rest)):
                        (i, j) = pool_rest[k]
                        pp = ppp.tile(
                            [C, SLAB, W], f16, name=f"ppl{k}", tag=f"pp{k % 3}"
                        )
                        nc.vector.tensor_scalar_mul(
                            pp, xin(i, j), wall[:, b, i * KW + j : i * KW + j + 1]
                        )
                        nc.gpsimd.tensor_tensor(out=pacc, in0=pp, in1=pacc, op=A.add)

                    nc.vector.tensor_tensor(
                        out=dacc[0], in0=dacc[1], in1=dacc[0], op=A.add
                    )

                    # prebuild next sample's diagonals mid-sample
                    if s == 4 and b + 1 < BPC:
                        diag_tiles[b + 1] = build_diags(b + 1)

                    # --- absorb partials into PSUM, evict via ScalarE ---
                    ot = outp.tile([C, SLAB, W], f16)
                    for q in range(NCHUNK):
                        r0 = q * CHUNK_ROWS
                        nc.tensor.matmul(
                            pst[q], eye_t,
                            dacc[0][:, r0 : r0 + CHUNK_ROWS, :],
                            start=False, stop=False,
                        )
                        nc.tensor.matmul(
                            pst[q], eye_t,
                            pacc[:, r0 : r0 + CHUNK_ROWS, :],
                            start=False, stop=True,
                        )
                        nc.scalar.copy(ot[:, r0 : r0 + CHUNK_ROWS, :], pst[q])
                    nc.sync.dma_start(out=o[b, :, y0 : y0 + SLAB, :], in_=ot)

    nc.compile()
    return nc


def _get_program():
    if "nc" not in _PROGRAM_CACHE:
        _PROGRAM_CACHE["nc"] = _build_program()
    return _PROGRAM_CACHE["nc"]


def _prep_inputs(inputs, kernels):
    """Host-side shard + layout transform. Returns per-core input maps."""
    xt = _PROGRAM_CACHE.get("xt")
    if xt is None:
        xt = np.zeros((B, C, HP, WP), np.float16)
        _PROGRAM_CACHE["xt"] = xt
    xt[:, :, PAD : PAD + H, PAD : PAD + W] = np.transpose(
        inputs, (0, 3, 1, 2)
    ).astype(np.float16)
    wt = np.ascontiguousarray(
        np.transpose(kernels, (0, 3, 1, 2)).reshape(B, C, KH * KW).astype(np.float32)
    )
    eye = np.eye(C, dtype=np.float16)
    in_maps = []
    for k in range(N_CORES):
        sl = slice(k * BPC, (k + 1) * BPC)
        in_maps.append({"x": xt[sl], "w": wt[sl], "eye": eye})
    return in_maps


def _gather_output(results):
    full = np.concatenate([r["o"] for r in results], axis=0)  # [B,C,H,W] f16
    return np.ascontiguousarray(
        np.transpose(full.astype(np.float32), (0, 2, 3, 1))
    )


def run_spmd(inputs, kernels, **spmd_kwargs):
    """Run on all 8 cores; returns (output, BassKernelResults)."""
    nc = _get_program()
    in_maps = _prep_inputs(np.asarray(inputs), np.asarray(kernels))
    res = run_bass_kernel_spmd(nc, in_maps, list(range(N_CORES)), **spmd_kwargs)
    return _gather_output(res.results), res


def kernel(inputs, kernels):
    out, _ = run_spmd(inputs, kernels)
    return out


# revision 13
# speedup vs baseline: 2.8981x; 1.0706x over previous
"""Per-sample depthwise 7x7 SAME cross-correlation on 8 trn2 NeuronCores.

Problem: inputs [32,128,128,128] (B,H,W,C), kernels [32,7,7,128] (B,KH,KW,C).
out[b,y,x,c] = sum_{i,j} inputs[b, y+i-3, x+j-3, c] * kernels[b,i,j,c]

Strategy (pure data parallel, 4 samples/core; all four compute engines used):
  - Host: channel-major fp16 zero-padded images [B, C, 134, 134]; per-tap
    fp32 scalars [B, C, 49]; fp16 output (converted back to fp32 on host).
    fp16 in/out halves DMA and enables the DVE 2-byte fast paths; products
    are exact to ~1e-3 which is far inside the 2e-2 gate.
  - The 49 taps are split across three concurrent compute paths:
      * TensorE (28 taps): out[c,:] += w_t[c] * xshift[c,:] as a matmul with
        a per-(sample,tap) DIAGONAL stationary matrix diag(kernels[b,:,i,j])
        and the shifted image as moving data; PSUM accumulates all taps in
        fp32. Diagonals are built on the fly (DVE: eye * w per-partition
        scalar, 49 small ops per sample). 512-px chunks (1 PSUM bank each).
      * DVE (14 taps): product passes via tensor_scalar_mul (4x fp16 mode)
        plus tensor_tensor adds (2x fp16 mode) into 2 independent fp16
        accumulator chains; cheaper than the fused scalar_tensor_tensor
        MAC which has no DVE fast modes.
      * GpSimd (7 taps): tensor_tensor adds of DVE-produced products into a
        third fp16 accumulator (the HW Pool engine only accepts
        TensorTensor-class ops, not TensorScalarPtr).
    The two side accumulators are absorbed into PSUM by identity matmuls
    and ScalarE evicts PSUM (fp32) into fp16 output tiles for DMA.
  - 16-row slabs: 4 PSUM banks per slab, double-buffered across slabs so
    eviction overlaps the next slab's matmuls.
"""

import numpy as np

import concourse.bass as bass
import concourse.tile as tile
from concourse import bacc, mybir
from concourse.bass_utils import run_bass_kernel_spmd

B, H, W, C = 32, 128, 128, 128
KH = KW = 7
PAD = 3
N_CORES = 8
BPC = B // N_CORES
HP, WP = H + 2 * PAD, W + 2 * PAD  # 134
SLAB = 16
N_SLABS = H // SLAB  # 8 per sample
ROWS = SLAB + 2 * PAD  # 22 input rows per slab
CHUNK_ROWS = 4  # 4*128 = 512 px = one PSUM bank
NCHUNK = SLAB // CHUNK_ROWS  # 4

# Tap split across engines (model-balanced: PE 241.9ns/chunk, DVE
# 611+1144ns/slab-pass, Pool 4158ns/add).
_ALL = [(i, j) for i in range(KH) for j in range(KW)]
N_PE, N_DVE, N_POOL = 27, 15, 7
PE_TAPS = _ALL[:N_PE]
DVE_TAPS = _ALL[N_PE : N_PE + N_DVE]
POOL_TAPS = _ALL[N_PE + N_DVE :]

f32 = mybir.dt.float32
f16 = mybir.dt.float16
A = mybir.AluOpType

_PROGRAM_CACHE = {}


def _build_program(repeat=1):
    nc = bacc.Bacc("TRN2", target_bir_lowering=False, debug=False)
    x_h = nc.dram_tensor("x", [BPC, C, HP, WP], f16, kind="ExternalInput")
    w_h = nc.dram_tensor("w", [BPC, C, KH * KW], f32, kind="ExternalInput")
    eye_h = nc.dram_tensor("eye", [C, C], f16, kind="ExternalInput")
    o_h = nc.dram_tensor("o", [BPC, C, H, W], f16, kind="ExternalOutput")
    x, w, eye, o = x_h.ap(), w_h.ap(), eye_h.ap(), o_h.ap()

    with tile.TileContext(nc) as tc:
        with (
            tc.tile_pool(name="const", bufs=1) as constp,
            tc.tile_pool(name="xp", bufs=3) as xp,
            tc.tile_pool(name="diagp", bufs=2) as diagp,
            tc.tile_pool(name="daccp", bufs=3) as daccp,
            tc.tile_pool(name="paccp", bufs=3) as paccp,
            tc.tile_pool(name="prodp", bufs=3) as prodp,
            tc.tile_pool(name="ppp", bufs=3) as ppp,
            tc.tile_pool(name="outp", bufs=3) as outp,
            tc.psum_pool(name="ps", bufs=2) as psp,
        ):
            eye_t = constp.tile([C, C], f16)
            wall = constp.tile([C, BPC, KH * KW], f32)
            nc.sync.dma_start(out=eye_t, in_=eye)
            for b in range(BPC):
                nc.sync.dma_start(out=wall[:, b, :], in_=w[b])

            def alloc_diags(b):
                return diagp.tile(
                    [C, KH * KW, C], f16, name=f"diag{b}", tag=f"diag{b % 2}"
                )

            def build_diags(b, dt, t0, t1):
                """Diagonal stationary matrices [t0,t1) for sample b (ScalarE)."""
                for t in range(t0, t1):
                    nc.scalar.mul(dt[:, t, :], eye_t, wall[:, b, t : t + 1])

            for rep in range(repeat):
                diag_tiles = {0: alloc_diags(0)}
                build_diags(0, diag_tiles[0], 0, KH * KW)

                for b, s in [
                    (b, s) for b in range(BPC) for s in range(N_SLABS)
                ]:
                    y0 = s * SLAB
                    xt = xp.tile([C, ROWS, WP], f16)
                    nc.sync.dma_start(out=xt, in_=x[b, :, y0 : y0 + ROWS, :])
                    dt = diag_tiles[b]

                    # --- PE diag taps, tap-outer for stationary reuse ---
                    pst = [
                        psp.tile([C, 512], f32, name=f"ps{q}", tag=f"ps{q}")
                        for q in range(NCHUNK)
                    ]
                    for ti, (i, j) in enumerate(PE_TAPS):
                        t = i * KW + j
                        for q in range(NCHUNK):
                            r0 = i + q * CHUNK_ROWS
                            rhs = xt[:, r0 : r0 + CHUNK_ROWS, j : j + W]
                            nc.tensor.matmul(
                                pst[q], dt[:, t, :], rhs,
                                start=(ti == 0), stop=False,
                            )

                    # --- DVE + Pool paths ---
                    pacc = paccp.tile([C, SLAB, W], f16, name="pacc", tag="pacc")
                    NCH = 3  # independent DVE chains (drain overlap on HW)
                    dacc = [
                        daccp.tile([C, SLAB, W], f16, name=f"dacc{ch}", tag=f"dacc{ch}")
                        for ch in range(NCH)
                    ]

                    def xin(i, j):
                        return xt[:, i : i + SLAB, j : j + W]

                    def wcol(i, j):
                        t = i * KW + j
                        return wall[:, b, t : t + 1]

                    # pool products come from ScalarE (DVE is the bottleneck);
                    # pool seed too, so Pool only ever runs TT adds.
                    (pi, pj) = POOL_TAPS[0]
                    nc.scalar.mul(pacc, xin(pi, pj), wcol(pi, pj))
                    for k, (i, j) in enumerate(POOL_TAPS[1:]):
                        pp = ppp.tile(
                            [C, SLAB, W], f16, name=f"pp{k}", tag=f"pp{k % 3}"
                        )
                        nc.scalar.mul(pp, xin(i, j), wcol(i, j))
                        nc.gpsimd.tensor_tensor(out=pacc, in0=pp, in1=pacc, op=A.add)

                    # DVE chains: tsmul products (4x mode) + TT adds (2x mode)
                    for ch in range(NCH):
                        (i, j) = DVE_TAPS[ch]
                        nc.vector.tensor_scalar_mul(dacc[ch], xin(i, j), wcol(i, j))
                    for k in range(NCH, N_DVE, NCH):
                        grp = [
                            (ch, DVE_TAPS[k + ch])
                            for ch in range(NCH)
                            if k + ch < N_DVE
                        ]
                        prods = []
                        for ch, (i, j) in grp:
                            pr = prodp.tile(
                                [C, SLAB, W], f16, name=f"pr{ch}", tag=f"pr{ch}"
                            )
                            nc.vector.tensor_scalar_mul(pr, xin(i, j), wcol(i, j))
                            prods.append((ch, pr))
                        for ch, pr in prods:
                            nc.vector.tensor_tensor(
                                out=dacc[ch], in0=pr, in1=dacc[ch], op=A.add
                            )
                    nc.vector.tensor_tensor(
                        out=dacc[0], in0=dacc[1], in1=dacc[0], op=A.add
                    )
                    nc.vector.tensor_tensor(
                        out=dacc[0], in0=dacc[2], in1=dacc[0], op=A.add
                    )

                    # prebuild next sample's diagonals spread over slabs 2-6
                    if 2 <= s <= 6 and b + 1 < BPC:
                        if s == 2:
                            diag_tiles[b + 1] = alloc_diags(b + 1)
                        build_diags(
                            b + 1, diag_tiles[b + 1],
                            (s - 2) * 10, min(49, (s - 1) * 10),
                        )

                    # --- absorb + evict, deferred one slab so the PE queue
                    # runs the NEXT slab's taps while this slab's side
                    # accumulators finish merging (hides the DVE tail) ---
                    def make_finish(pst, dacc0, pacc, b, y0):
                        def fin():
                            ot = outp.tile([C, SLAB, W], f16, name="ot")
                            for q in range(NCHUNK):
                                r0 = q * CHUNK_ROWS
                                nc.tensor.matmul(
                                    pst[q], eye_t,
                                    dacc0[:, r0 : r0 + CHUNK_ROWS, :],
                                    start=False, stop=False,
                                )
                                nc.tensor.matmul(
                                    pst[q], eye_t,
                                    pacc[:, r0 : r0 + CHUNK_ROWS, :],
                                    start=False, stop=True,
                                )
                                nc.scalar.copy(ot[:, r0 : r0 + CHUNK_ROWS, :], pst[q])
                            nc.sync.dma_start(out=o[b, :, y0 : y0 + SLAB, :], in_=ot)

                        return fin

                    make_finish(pst, dacc[0], pacc, b, y0)()

    nc.compile()
    return nc


def _get_program():
    if "nc" not in _PROGRAM_CACHE:
        _PROGRAM_CACHE["nc"] = _build_program()
    return _PROGRAM_CACHE["nc"]


def _prep_inputs(inputs, kernels):
    """Host-side shard + layout transform. Returns per-core input maps."""
    xt = _PROGRAM_CACHE.get("xt")
    if xt is None:
        xt = np.zeros((B, C, HP, WP), np.float16)
        _PROGRAM_CACHE["xt"] = xt
    xt[:, :, PAD : PAD + H, PAD : PAD + W] = np.transpose(
        inputs, (0, 3, 1, 2)
    ).astype(np.float16)
    wt = np.ascontiguousarray(
        np.transpose(kernels, (0, 3, 1, 2)).reshape(B, C, KH * KW).astype(np.float32)
    )
    eye = np.eye(C, dtype=np.float16)
    in_maps = []
    for k in range(N_CORES):
        sl = slice(k * BPC, (k + 1) * BPC)
        in_maps.append({"x": xt[sl], "w": wt[sl], "eye": eye})
    return in_maps


def _gather_output(results):
    full = np.concatenate([r["o"] for r in results], axis=0)  # [B,C,H,W] f16
    return np.ascontiguousarray(
        np.transpose(full.astype(np.float32), (0, 2, 3, 1))
    )


def run_spmd(inputs, kernels, **spmd_kwargs):
    """Run on all 8 cores; returns (output, BassKernelResults)."""
    nc = _get_program()
    in_maps = _prep_inputs(np.asarray(inputs), np.asarray(kernels))
    res = run_bass_kernel_spmd(nc, in_maps, list(range(N_CORES)), **spmd_kwargs)
    return _gather_output(res.results), res


def kernel(inputs, kernels):
    out, _ = run_spmd(inputs, kernels)
    return out
